# revision 1
# baseline (speedup 1.0000x reference)
"""LinkPredictor (GNN edge scorer) Bass kernel for 8 Trainium2 NeuronCores.

score[e] = W2 @ relu(W1 @ [h[src[e]]; h[dst[e]]] + b1) + b2

Strategy (pure data parallel over edges, per sharding hint):
  - shard E=1.6M edges across 8 cores (200k each, padded to 391*512)
  - replicate h and MLP weights
  - per 512-edge tile: indirect-DMA gather of h rows (512B each),
    PE transpose to [feat, edge] layout, fp32r matmuls for both layers,
    ScalarE fused bias+relu, DVE final bias add.
"""

import numpy as np

N_NODES = 100000
N_EDGES = 1600000
D = 128
H = 256
N_CORES = 8
E_PER_CORE = N_EDGES // N_CORES  # 200000
K_SUB = 4                        # 128-edge subblocks per tile
TILE_E = 128 * K_SUB             # 512 edges per tile
N_TILES = (E_PER_CORE + TILE_E - 1) // TILE_E  # 391
E_PAD = N_TILES * TILE_E         # 200192

_cache = {}


def _build_nc():
    from contextlib import ExitStack

    import concourse.bass as bass
    import concourse.tile as tile
    from concourse import bacc, mybir
    from concourse.masks import make_identity

    f32 = mybir.dt.float32
    f32r = mybir.dt.float32r
    i32 = mybir.dt.int32

    nc = bacc.Bacc("TRN2", target_bir_lowering=False, debug=False)

    h_d = nc.dram_tensor("h", [N_NODES, D], f32, kind="ExternalInput")
    src_d = nc.dram_tensor("srcT", [N_TILES, 128, K_SUB], i32, kind="ExternalInput")
    dst_d = nc.dram_tensor("dstT", [N_TILES, 128, K_SUB], i32, kind="ExternalInput")
    w1t_d = nc.dram_tensor("W1T", [2 * D, H], f32, kind="ExternalInput")  # W1_w.T
    b1_d = nc.dram_tensor("b1", [H], f32, kind="ExternalInput")
    w2_d = nc.dram_tensor("W2", [H], f32, kind="ExternalInput")
    b2_d = nc.dram_tensor("b2", [1, 1], f32, kind="ExternalInput")
    out_d = nc.dram_tensor("out", [N_TILES, 1, TILE_E], f32, kind="ExternalOutput")

    relu = mybir.ActivationFunctionType.Relu

    with tile.TileContext(nc) as tc, ExitStack() as ctx:
        const = ctx.enter_context(tc.tile_pool(name="const", bufs=1))
        idxp = ctx.enter_context(tc.tile_pool(name="idx", bufs=4))
        gp = ctx.enter_context(tc.tile_pool(name="gather", bufs=4))
        tsp = ctx.enter_context(tc.tile_pool(name="tsb", bufs=2))
        rp = ctx.enter_context(tc.tile_pool(name="relu", bufs=2))
        scp = ctx.enter_context(tc.tile_pool(name="score", bufs=4))
        ts_ps = ctx.enter_context(tc.tile_pool(name="ts_ps", bufs=1, space="PSUM"))
        mm_ps = ctx.enter_context(tc.tile_pool(name="mm_ps", bufs=2, space="PSUM"))
        sc_ps = ctx.enter_context(tc.tile_pool(name="sc_ps", bufs=2, space="PSUM"))

        # --- constants ---
        ident = const.tile([128, 128], f32)
        make_identity(nc, ident[:])
        w1_f0 = const.tile([128, H], f32)   # W1T rows 0:128  (src features)
        w1_f1 = const.tile([128, H], f32)   # W1T rows 128:256 (dst features)
        nc.sync.dma_start(w1_f0[:], w1t_d[0:128, :])
        nc.sync.dma_start(w1_f1[:], w1t_d[128:256, :])
        b1t = const.tile([128, 2], f32)
        nc.sync.dma_start(b1t[:, 0:1], b1_d[0:128, None])
        nc.sync.dma_start(b1t[:, 1:2], b1_d[128:256, None])
        w2t = const.tile([128, 2], f32)
        nc.sync.dma_start(w2t[:, 0:1], w2_d[0:128, None])
        nc.sync.dma_start(w2t[:, 1:2], w2_d[128:256, None])
        b2t = const.tile([1, 1], f32)
        nc.sync.dma_start(b2t[:], b2_d[:])
        w1r_f0 = const.tile([128, H], f32r)
        w1r_f1 = const.tile([128, H], f32r)
        w2r = const.tile([128, 2], f32r)
        nc.vector.tensor_copy(w1r_f0[:], w1_f0[:])
        nc.vector.tensor_copy(w1r_f1[:], w1_f1[:])
        nc.vector.tensor_copy(w2r[:], w2t[:])

        for t in range(N_TILES):
            # --- indices (host pre-permuted so (p, j) = edge j*128+p) ---
            is_ = idxp.tile([128, K_SUB], i32, tag="is")
            id_ = idxp.tile([128, K_SUB], i32, tag="id")
            nc.sync.dma_start(is_[:], src_d[t])
            nc.sync.dma_start(id_[:], dst_d[t])

            # --- gather h rows: gs[p, j*128:(j+1)*128] = h[is_[p, j], :] ---
            gs = gp.tile([128, TILE_E], f32, tag="gs")
            gd = gp.tile([128, TILE_E], f32, tag="gd")
            for j in range(K_SUB):
                sl = slice(j * 128, (j + 1) * 128)
                nc.gpsimd.indirect_dma_start(
                    out=gs[:, sl], out_offset=None, in_=h_d[:],
                    in_offset=bass.IndirectOffsetOnAxis(ap=is_[:, j:j + 1], axis=0))
                nc.gpsimd.indirect_dma_start(
                    out=gd[:, sl], out_offset=None, in_=h_d[:],
                    in_offset=bass.IndirectOffsetOnAxis(ap=id_[:, j:j + 1], axis=0))

            # --- PE transpose each [128e, 128f] subblock -> [128f, 128e] ---
            tps = ts_ps.tile([128, TILE_E], f32, tag="tps")
            tpd = ts_ps.tile([128, TILE_E], f32, tag="tpd")
            for j in range(K_SUB):
                sl = slice(j * 128, (j + 1) * 128)
                nc.tensor.matmul(tps[:, sl], lhsT=gs[:, sl], rhs=ident[:],
                                 is_transpose=True, start=(j == 0), stop=(j == K_SUB - 1))
            for j in range(K_SUB):
                sl = slice(j * 128, (j + 1) * 128)
                nc.tensor.matmul(tpd[:, sl], lhsT=gd[:, sl], rhs=ident[:],
                                 is_transpose=True, start=(j == 0), stop=(j == K_SUB - 1))

            tss = tsp.tile([128, TILE_E], f32r, tag="tss")
            tsd = tsp.tile([128, TILE_E], f32r, tag="tsd")
            nc.vector.tensor_copy(tss[:], tps[:])
            nc.vector.tensor_copy(tsd[:], tpd[:])

            # --- layer 1: r[m, e] = sum_f W1T[f, m] * x[f, e]  (fp32r) ---
            r0 = mm_ps.tile([128, TILE_E], f32, tag="r0")
            r1 = mm_ps.tile([128, TILE_E], f32, tag="r1")
            nc.tensor.matmul(r0[:], lhsT=w1r_f0[:, 0:128],
                             rhs=tss[:], start=True, stop=False)
            nc.tensor.matmul(r0[:], lhsT=w1r_f1[:, 0:128],
                             rhs=tsd[:], start=False, stop=True)
            nc.tensor.matmul(r1[:], lhsT=w1r_f0[:, 128:256],
                             rhs=tss[:], start=True, stop=False)
            nc.tensor.matmul(r1[:], lhsT=w1r_f1[:, 128:256],
                             rhs=tsd[:], start=False, stop=True)

            # --- bias + relu (ScalarE, psum -> sbuf) ---
            R0 = rp.tile([128, TILE_E], f32r, tag="R0")
            R1 = rp.tile([128, TILE_E], f32r, tag="R1")
            nc.scalar.activation(R0[:], r0[:], relu, bias=b1t[:, 0:1], scale=1.0)
            nc.scalar.activation(R1[:], r1[:], relu, bias=b1t[:, 1:2], scale=1.0)

            # --- layer 2: sc[0, e] = sum_h W2[h] * R[h, e] ---
            sc = sc_ps.tile([1, TILE_E], f32, tag="sc")
            nc.tensor.matmul(sc[:], lhsT=w2r[:, 0:1],
                             rhs=R0[:], start=True, stop=False)
            nc.tensor.matmul(sc[:], lhsT=w2r[:, 1:2],
                             rhs=R1[:], start=False, stop=True)

            # --- + b2, psum -> sbuf, store ---
            sco = scp.tile([1, TILE_E], f32, tag="sco")
            nc.vector.tensor_scalar(out=sco[:], in0=sc[:], scalar1=b2t[:],
                                    scalar2=None, op0=mybir.AluOpType.add)
            nc.sync.dma_start(out_d[t], sco[:])

    nc.compile()
    return nc


def _get_nc():
    if "nc" not in _cache:
        _cache["nc"] = _build_nc()
    return _cache["nc"]


def _prep_idx(idx_i64):
    """Per-core index array -> [N_TILES, 128, K_SUB] int32 so that the
    device tile (p, j) holds edge j*128 + p (contiguous device output)."""
    a = np.zeros(E_PAD, dtype=np.int32)
    a[: idx_i64.shape[0]] = idx_i64.astype(np.int32)
    return np.ascontiguousarray(
        a.reshape(N_TILES, K_SUB, 128).transpose(0, 2, 1))


def _make_runner(nc):
    """Replicates bass2jax.run_bass_via_pjrt's multi-core shard_map path but
    returns a reusable jitted callable so repeated (timed) runs are possible."""
    import jax
    import numpy as _np
    from jax.sharding import Mesh, PartitionSpec
    from jax.experimental.shard_map import shard_map

    import concourse.mybir as mybir
    from concourse.bass2jax import (
        _bass_exec_p, install_neuronx_cc_hook, partition_id_tensor)

    install_neuronx_cc_hook()

    partition_name = (
        nc.partition_id_tensor.name if nc.partition_id_tensor else None)
    in_names, out_names, out_avals, zero_outs = [], [], [], []
    for alloc in nc.m.functions[0].allocations:
        if not isinstance(alloc, mybir.MemoryLocationSet):
            continue
        name = alloc.memorylocations[0].name
        if alloc.kind == "ExternalInput":
            if name != partition_name:
                in_names.append(name)
        elif alloc.kind == "ExternalOutput":
            out_names.append(name)
            shape = tuple(alloc.tensor_shape)
            dtype = mybir.dt.np(alloc.dtype)
            out_avals.append(jax.core.ShapedArray(shape, dtype))
            zero_outs.append(_np.zeros(shape, dtype))
    n_params = len(in_names)
    n_outs = len(out_avals)
    all_names = in_names + out_names
    if partition_name is not None:
        all_names = all_names + [partition_name]
    donate = tuple(range(n_params, n_params + n_outs))

    def _body(*args):
        operands = list(args)
        if partition_name is not None:
            operands.append(partition_id_tensor())
        outs = _bass_exec_p.bind(
            *operands,
            out_avals=tuple(out_avals),
            in_names=tuple(all_names),
            out_names=tuple(out_names),
            lowering_input_output_aliases=(),
            sim_require_finite=True,
            sim_require_nnan=True,
            nc=nc,
        )
        return tuple(outs)

    devices = jax.devices()[:N_CORES]
    mesh = Mesh(np.asarray(devices), ("core",))
    sharded = jax.jit(
        shard_map(_body, mesh=mesh,
                  in_specs=(PartitionSpec("core"),) * (n_params + n_outs),
                  out_specs=(PartitionSpec("core"),) * n_outs,
                  check_rep=False),
        donate_argnums=donate, keep_unused=True)
    return sharded, in_names, out_names, out_avals, zero_outs


def kernel(h, src, dst, W1_w, W1_b, W2_w, W2_b, _time_iters=0):
    import jax

    nc = _get_nc()

    h = np.ascontiguousarray(np.asarray(h, dtype=np.float32))
    w1t = np.ascontiguousarray(np.asarray(W1_w, dtype=np.float32).T)
    b1 = np.ascontiguousarray(np.asarray(W1_b, dtype=np.float32))
    w2 = np.ascontiguousarray(np.asarray(W2_w, dtype=np.float32).reshape(H))
    b2 = np.asarray(W2_b, dtype=np.float32).reshape(1, 1)

    in_maps = []
    for c in range(N_CORES):
        sl = slice(c * E_PER_CORE, (c + 1) * E_PER_CORE)
        in_maps.append({
            "h": h,
            "srcT": _prep_idx(np.asarray(src[sl])),
            "dstT": _prep_idx(np.asarray(dst[sl])),
            "W1T": w1t,
            "b1": b1,
            "W2": w2,
            "b2": b2,
        })

    if "runner" not in _cache:
        _cache["runner"] = _make_runner(nc)
    sharded, in_names, out_names, out_avals, zero_outs = _cache["runner"]

    concat_in = [
        np.concatenate([in_maps[c][name] for c in range(N_CORES)], axis=0)
        for name in in_names
    ]
    concat_zeros = [
        np.zeros((N_CORES * z.shape[0], *z.shape[1:]), z.dtype) for z in zero_outs
    ]
    out_arrs = sharded(*concat_in, *concat_zeros)
    jax.block_until_ready(out_arrs)

    if _time_iters > 0:
        import time
        dev_in = [jax.device_put(a) for a in concat_in]
        # warmup already done above; time with pre-staged inputs
        times = []
        for _ in range(_time_iters):
            zs = [np.zeros((N_CORES * z.shape[0], *z.shape[1:]), z.dtype)
                  for z in zero_outs]
            t0 = time.perf_counter()
            o = sharded(*dev_in, *zs)
            jax.block_until_ready(o)
            times.append(time.perf_counter() - t0)
        kernel.exec_times_s = times

    oi = out_names.index("out")
    full = np.asarray(out_arrs[oi]).reshape(N_CORES, *out_avals[oi].shape)
    out = np.concatenate([full[c].reshape(-1)[:E_PER_CORE] for c in range(N_CORES)])
    return out.astype(np.float32)

